# revision 1
# baseline (speedup 1.0000x reference)
"""DistMult scoring kernel for Trainium2 (8 NeuronCores, Bass/Tile).

reference computation:
    rel = rel_embeds[rel_ids]                      # [B, D] gather
    scores = sum(head * rel * tail, axis=-1)       # [B]
    pos = min(scores[:n_pos], upper_bound)
    neg = max(scores[n_pos:], lower_bound)
    out = sigmoid(concat(pos, neg))

Sharding: data-parallel over B. Core c owns rows [c*65536, (c+1)*65536).
Within a core, local row r maps to (partition p, column t) with r = p*512 + t,
which makes every stream DMA contiguous per partition and lets the final
[128, 512] score tile be stored with a single contiguous DMA.

The pos/neg split falls on a core boundary (131072 = 2 * 65536), handled
data-parallel by feeding cores +/-inf padded bounds:
    out = sigmoid(max(min(scores, ub), lb))
with ub=+inf for neg cores and lb=-inf for pos cores.
"""

import sys

for _p in ("/opt/trn_rl_repo",):
    if _p not in sys.path:
        sys.path.insert(0, _p)

import numpy as np

import concourse.bacc as bacc
import concourse.bass as bass
import concourse.mybir as mybir
import concourse.tile as tile
from concourse.bass_utils import run_bass_kernel_spmd

N_POS = 131072
N_NEG = 393216
B = N_POS + N_NEG  # 524288
D = 256
NUM_REL = 500
NCORES = 8
ROWS = B // NCORES  # 65536 rows per core
P = 128
T = ROWS // P  # 512 tiles of 128 rows; local row = p*T + t
GROUP = 8  # tiles per loop iteration
NG = T // GROUP  # 64 iterations

# stream dtype for head/tail/rel table ("f32" or "bf16")
STREAM_DT = "bf16"


def build_program(stream_dt: str = STREAM_DT):
    sdt = mybir.dt.float32 if stream_dt == "f32" else mybir.dt.bfloat16
    f32 = mybir.dt.float32
    i32 = mybir.dt.int32
    mult = mybir.AluOpType.mult
    add = mybir.AluOpType.add

    nc = bacc.Bacc(
        "TRN2", target_bir_lowering=False, debug=False, num_devices=NCORES
    )
    h = nc.declare_dram_parameter("h", [ROWS, D], sdt, isOutput=False)
    t_ = nc.declare_dram_parameter("t", [ROWS, D], sdt, isOutput=False)
    # pair ids: ids[p, 2u]*NUM_REL + ids[p, 2u+1], [ROWS//2] int32
    ids = nc.declare_dram_parameter("ids", [ROWS // 2], i32, isOutput=False)
    ub = nc.declare_dram_parameter("ub", [ROWS], f32, isOutput=False)
    lb = nc.declare_dram_parameter("lb", [ROWS], f32, isOutput=False)
    # pair table: row i*NUM_REL+j = concat(table[i], table[j])
    table = nc.declare_dram_parameter(
        "table", [NUM_REL * NUM_REL, 2 * D], sdt, isOutput=False
    )
    out = nc.declare_dram_parameter("out", [ROWS], f32, isOutput=True)

    h_v = h[:].rearrange("(p t) d -> p t d", p=P)
    t_v = t_[:].rearrange("(p t) d -> p t d", p=P)
    ids_v = ids[:].rearrange("(p t) -> p t", p=P)
    ub_v = ub[:].rearrange("(p t) -> p t", p=P)
    lb_v = lb[:].rearrange("(p t) -> p t", p=P)
    out_v = out[:].rearrange("(p t) -> p t", p=P)

    with tile.TileContext(nc) as tc:
        with (
            tc.tile_pool(name="io", bufs=1) as io_pool,
            tc.tile_pool(name="stream", bufs=4) as spool,
            tc.tile_pool(name="rpool", bufs=8) as rpool,
            tc.tile_pool(name="scratch", bufs=2) as qpool,
        ):
            ids_all = io_pool.tile([P, T // 2], i32)
            nc.sync.dma_start(out=ids_all[:], in_=ids_v)
            scores = io_pool.tile([P, T], f32)

            for g in range(NG):
                t0 = g * GROUP
                htile = spool.tile([P, GROUP * D], sdt, tag="h")
                ttile = spool.tile([P, GROUP * D], sdt, tag="t")
                rtile = rpool.tile([P, GROUP * D], sdt, tag="r")
                nc.sync.dma_start(
                    out=htile[:].rearrange("p (g d) -> p g d", g=GROUP),
                    in_=h_v[:, t0 : t0 + GROUP, :],
                )
                nc.sync.dma_start(
                    out=ttile[:].rearrange("p (g d) -> p g d", g=GROUP),
                    in_=t_v[:, t0 : t0 + GROUP, :],
                )
                u0 = t0 // 2
                for u in range(GROUP // 2):
                    nc.gpsimd.indirect_dma_start(
                        out=rtile[:, u * 2 * D : (u + 1) * 2 * D],
                        out_offset=None,
                        in_=table[:],
                        in_offset=bass.IndirectOffsetOnAxis(
                            ap=ids_all[:, u0 + u : u0 + u + 1], axis=0
                        ),
                    )
                q = qpool.tile([P, GROUP * D], sdt, tag="q")
                s = qpool.tile([P, GROUP * D], sdt, tag="s")
                nc.vector.tensor_tensor(
                    out=q[:], in0=htile[:], in1=ttile[:], op=mult
                )
                for gi in range(GROUP):
                    seg = slice(gi * D, (gi + 1) * D)
                    nc.vector.scalar_tensor_tensor(
                        out=s[:, seg],
                        in0=q[:, seg],
                        scalar=1.0,
                        in1=rtile[:, seg],
                        op0=mult,
                        op1=mult,
                        accum_out=scores[:, t0 + gi : t0 + gi + 1],
                    )

            # tail: clamp + sigmoid + store
            ubt = io_pool.tile([P, T], f32)
            lbt = io_pool.tile([P, T], f32)
            nc.sync.dma_start(out=ubt[:], in_=ub_v)
            nc.sync.dma_start(out=lbt[:], in_=lb_v)
            clip1 = io_pool.tile([P, T], f32)
            clip2 = io_pool.tile([P, T], f32)
            nc.vector.tensor_tensor(
                out=clip1[:], in0=scores[:], in1=ubt[:], op=mybir.AluOpType.min
            )
            nc.vector.tensor_tensor(
                out=clip2[:], in0=clip1[:], in1=lbt[:], op=mybir.AluOpType.max
            )
            sig = io_pool.tile([P, T], f32)
            nc.scalar.activation(
                out=sig[:], in_=clip2[:], func=mybir.ActivationFunctionType.Sigmoid
            )
            nc.sync.dma_start(out=out_v, in_=sig[:])

    nc.compile()
    return nc


def make_in_maps(inputs: dict, stream_dt: str = STREAM_DT):
    np_sdt = np.float32 if stream_dt == "f32" else None
    import ml_dtypes

    if np_sdt is None:
        np_sdt = ml_dtypes.bfloat16

    head = np.asarray(inputs["head_embeds"], dtype=np.float32)
    tail = np.asarray(inputs["tail_embeds"], dtype=np.float32)
    rel_ids = np.asarray(inputs["rel_ids"]).astype(np.int32)
    lower = np.asarray(inputs["lower_bound"], dtype=np.float32)
    upper = np.asarray(inputs["upper_bound"], dtype=np.float32)
    table1 = np.asarray(inputs["rel_embeds"], dtype=np.float32).astype(np_sdt)

    head = head.astype(np_sdt)
    tail = tail.astype(np_sdt)

    # pair table: row i*NUM_REL+j = [table[i] | table[j]]
    table = np.empty((NUM_REL * NUM_REL, 2 * D), dtype=np_sdt)
    table[:, :D] = np.repeat(table1, NUM_REL, axis=0)
    table[:, D:] = np.tile(table1, (NUM_REL, 1))

    pos_inf = np.full(ROWS, np.inf, dtype=np.float32)
    neg_inf = np.full(ROWS, -np.inf, dtype=np.float32)

    in_maps = []
    for c in range(NCORES):
        lo = c * ROWS
        hi = lo + ROWS
        if hi <= N_POS:
            ub_c = upper[lo:hi]
            lb_c = neg_inf
        else:
            assert lo >= N_POS
            ub_c = pos_inf
            lb_c = lower[lo - N_POS : hi - N_POS]
        # pair ids in (p, u) layout: local row r = p*T + t; pairs along t
        ids_c = rel_ids[lo:hi].reshape(P, T // 2, 2).astype(np.int64)
        pair_ids = (ids_c[:, :, 0] * NUM_REL + ids_c[:, :, 1]).astype(np.int32)
        in_maps.append(
            {
                "h": np.ascontiguousarray(head[lo:hi]),
                "t": np.ascontiguousarray(tail[lo:hi]),
                "ids": np.ascontiguousarray(pair_ids.reshape(-1)),
                "ub": np.ascontiguousarray(ub_c),
                "lb": np.ascontiguousarray(lb_c),
                "table": table,
            }
        )
    return in_maps


def kernel(**inputs) -> np.ndarray:
    nc = build_program(STREAM_DT)
    in_maps = make_in_maps(inputs, STREAM_DT)
    res = run_bass_kernel_spmd(nc, in_maps, list(range(NCORES)))
    return np.concatenate([res.results[c]["out"] for c in range(NCORES)])



# revision 3
# speedup vs baseline: 1.9134x; 1.9134x over previous
"""DistMult scoring kernel for Trainium2 (8 NeuronCores, Bass/Tile).

reference computation:
    rel = rel_embeds[rel_ids]                      # [B, D] gather
    scores = sum(head * rel * tail, axis=-1)       # [B]
    pos = min(scores[:n_pos], upper_bound)
    neg = max(scores[n_pos:], lower_bound)
    out = sigmoid(concat(pos, neg))

Strategy (matmul-scored, sorted batches):
  * Host sorts all B rows by rel_id and pads each rel group to a multiple
    of 128 rows, so every 128-row "batch" uses exactly ONE rel vector.
    Batches are dealt contiguously to 8 cores; outputs are unpermuted on
    the host (row order is free to choose since the final gather is ours).
  * Rows are laid out TRANSPOSED on device: d on partitions (2 chunks of
    128), rows on the free axis. h and t are stored int8 (x32) in DRAM and
    upcast to bf16 during the SWDGE DMA (halves HBM traffic; DVE needs
    16-bit for its 2x mode).
  * DVE does a single tensor_tensor pass q = h*t (bf16, 2x mode).
  * Per batch b, the score reduction over d is TWO matmuls on the (idle)
    tensor engine: lhsT = q_chunk[:, b*128:(b+1)*128] (stationary),
    rhs = g[:, 2b+c] (the batch's rel vector chunk, moving, N=1),
    accumulating into psum[:, b]. Measured issue rate ~33ns per pair.
  * psum[i, b] = 1024 * score(row 128b + i). Clamp against host-prescaled
    bounds (x1024, +/-inf on the inactive side), then sigmoid via the
    scalar engine with scale=2^-10. One output DMA; host unpermutes.
"""

import sys

for _p in ("/opt/trn_rl_repo",):
    if _p not in sys.path:
        sys.path.insert(0, _p)

import numpy as np

import concourse.bacc as bacc
import concourse.bass as bass
import concourse.mybir as mybir
import concourse.tile as tile
from concourse.bass_utils import run_bass_kernel_spmd

N_POS = 131072
N_NEG = 393216
B = N_POS + N_NEG  # 524288
D = 256
NUM_REL = 500
NCORES = 8
P = 128
W = 128          # rows per batch (one rel vector per batch)
GB = 64          # batches per group (64*128 = 8192 rows per group-chunk tile)
HSCALE = 32.0    # h, t int8 quantization scale; scores come out x1024
SSCALE = float(HSCALE * HSCALE)


def build_program(nb: int):
    """nb: batches per core (each 128 rows, single rel)."""
    f32 = mybir.dt.float32
    i8 = mybir.dt.int8
    bf = mybir.dt.bfloat16
    mult = mybir.AluOpType.mult

    rows = nb * W
    ngroups = (nb + GB - 1) // GB

    nc = bacc.Bacc(
        "TRN2", target_bir_lowering=False, debug=False, num_devices=NCORES
    )
    # transposed int8 streams: [chunk, d, row]
    h8 = nc.declare_dram_parameter("h8", [2, P, rows], i8, isOutput=False)
    t8 = nc.declare_dram_parameter("t8", [2, P, rows], i8, isOutput=False)
    # per-batch rel vectors: g[d, 2b+c] = rel_vec(b)[128c + d]
    g = nc.declare_dram_parameter("g", [P, 2 * nb], bf, isOutput=False)
    ub = nc.declare_dram_parameter("ub", [P, nb], f32, isOutput=False)
    lb = nc.declare_dram_parameter("lb", [P, nb], f32, isOutput=False)
    out = nc.declare_dram_parameter("out", [P, nb], f32, isOutput=True)

    with tile.TileContext(nc) as tc:
        with (
            tc.tile_pool(name="io", bufs=1) as io_pool,
            tc.tile_pool(name="stream", bufs=2) as spool,
            tc.tile_pool(name="psum", bufs=4, space="PSUM") as psum_pool,
            tc.tile_pool(name="scratch", bufs=2) as qpool,
        ):
            gt = io_pool.tile([P, 2 * nb], bf)
            nc.sync.dma_start(out=gt[:], in_=g[:])
            ubt = io_pool.tile([P, nb], f32)
            nc.sync.dma_start(out=ubt[:], in_=ub[:])
            lbt = io_pool.tile([P, nb], f32)
            nc.sync.dma_start(out=lbt[:], in_=lb[:])
            scores = io_pool.tile([P, nb], f32)

            for gi in range(ngroups):
                b0 = gi * GB
                gb = min(GB, nb - b0)
                r0 = b0 * W
                gw = gb * W
                ht = [None, None]
                tt = [None, None]
                for c in range(2):
                    ht[c] = spool.tile([P, GB * W], bf, tag=f"h{c}", name=f"ht{c}")
                    tt[c] = spool.tile([P, GB * W], bf, tag=f"t{c}", name=f"tt{c}")
                    nc.gpsimd.dma_start(
                        out=ht[c][:, :gw], in_=h8[c, :, r0 : r0 + gw]
                    )
                    nc.gpsimd.dma_start(
                        out=tt[c][:, :gw], in_=t8[c, :, r0 : r0 + gw]
                    )
                # q = h * t, in place into the h tile (bf16, 2x mode)
                for c in range(2):
                    nc.vector.tensor_tensor(
                        out=ht[c][:, :gw], in0=ht[c][:, :gw], in1=tt[c][:, :gw],
                        op=mult,
                    )
                # per-batch score reduction on the tensor engine
                ps = psum_pool.tile([P, GB], f32, tag="ps")
                for b in range(gb):
                    for c in range(2):
                        nc.tensor.matmul(
                            out=ps[:, b : b + 1],
                            lhsT=ht[c][:, b * W : (b + 1) * W],
                            rhs=gt[:, 2 * (b0 + b) + c : 2 * (b0 + b) + c + 1],
                            start=(c == 0),
                            stop=(c == 1),
                        )
                # clamp into the persistent scores tile
                clip = qpool.tile([P, GB], f32, tag="clip")
                nc.vector.tensor_tensor(
                    out=clip[:, :gb], in0=ps[:, :gb], in1=ubt[:, b0 : b0 + gb],
                    op=mybir.AluOpType.min,
                )
                nc.vector.tensor_tensor(
                    out=scores[:, b0 : b0 + gb], in0=clip[:, :gb],
                    in1=lbt[:, b0 : b0 + gb], op=mybir.AluOpType.max,
                )

            sig = io_pool.tile([P, nb], f32)
            nc.scalar.activation(
                out=sig[:], in_=scores[:],
                func=mybir.ActivationFunctionType.Sigmoid,
                scale=1.0 / SSCALE,
            )
            nc.sync.dma_start(out=out[:], in_=sig[:])

    nc.compile()
    return nc


def make_in_maps(inputs: dict):
    import ml_dtypes

    bf16 = ml_dtypes.bfloat16

    head = np.asarray(inputs["head_embeds"], dtype=np.float32)
    tail = np.asarray(inputs["tail_embeds"], dtype=np.float32)
    rel_ids = np.asarray(inputs["rel_ids"]).astype(np.int64)
    lower = np.asarray(inputs["lower_bound"], dtype=np.float32)
    upper = np.asarray(inputs["upper_bound"], dtype=np.float32)
    table = np.asarray(inputs["rel_embeds"], dtype=np.float32)

    # --- sort rows by rel id, pad each rel group to a multiple of W rows
    order = np.argsort(rel_ids, kind="stable")
    sorted_ids = rel_ids[order]
    counts = np.bincount(sorted_ids, minlength=NUM_REL)
    padded = ((counts + W - 1) // W) * W
    total_batches = int(padded.sum()) // W
    nb = -(-total_batches // NCORES)  # ceil
    nbatch_total = nb * NCORES

    # row_src[j] = original row index or -1 (pad); batch_rel[b] = rel id
    row_src = np.full(nbatch_total * W, -1, dtype=np.int64)
    batch_rel = np.zeros(nbatch_total, dtype=np.int64)
    src_ofs = 0
    dst_ofs = 0
    bidx = 0
    for k in range(NUM_REL):
        n = int(counts[k])
        pn = int(padded[k])
        if pn == 0:
            continue
        row_src[dst_ofs : dst_ofs + n] = order[src_ofs : src_ofs + n]
        batch_rel[bidx : bidx + pn // W] = k
        src_ofs += n
        dst_ofs += pn
        bidx += pn // W

    # --- quantize h, t to int8 (x32) and build per-core transposed streams
    h8 = np.clip(np.round(head * HSCALE), -127, 127).astype(np.int8)
    t8 = np.clip(np.round(tail * HSCALE), -127, 127).astype(np.int8)
    src = row_src.copy()
    pad_mask = src < 0
    src[pad_mask] = 0
    h8_s = h8[src]
    t8_s = t8[src]
    h8_s[pad_mask] = 0
    t8_s[pad_mask] = 0

    # bounds in score space (x1024), +/-inf on the inactive side
    ubf = np.full(nbatch_total * W, np.inf, dtype=np.float32)
    lbf = np.full(nbatch_total * W, -np.inf, dtype=np.float32)
    pos_rows = (row_src >= 0) & (row_src < N_POS)
    neg_rows = row_src >= N_POS
    ubf[pos_rows] = upper[row_src[pos_rows]] * SSCALE
    lbf[neg_rows] = lower[row_src[neg_rows] - N_POS] * SSCALE

    table_bf = table.astype(bf16)

    rows = nb * W
    in_maps = []
    for c in range(NCORES):
        r0 = c * rows
        r1 = r0 + rows
        # [rows, 256] -> [256, rows] -> [2, 128, rows]
        hc = np.ascontiguousarray(h8_s[r0:r1].T).reshape(2, P, rows)
        tc_ = np.ascontiguousarray(t8_s[r0:r1].T).reshape(2, P, rows)
        # g[d, 2b+c] = table[rel(b), 128c+d]
        rel_c = batch_rel[c * nb : (c + 1) * nb]
        gc = table_bf[rel_c].reshape(nb, 2, P)  # [b, chunk, d]
        gc = np.ascontiguousarray(gc.transpose(2, 0, 1).reshape(P, 2 * nb))
        # bounds laid [i, b]: row j = 128b + i
        ub_c = np.ascontiguousarray(ubf[r0:r1].reshape(nb, W).T)
        lb_c = np.ascontiguousarray(lbf[r0:r1].reshape(nb, W).T)
        in_maps.append(
            {"h8": hc, "t8": tc_, "g": gc, "ub": ub_c, "lb": lb_c}
        )
    return in_maps, nb, row_src


def assemble_output(results, nb: int, row_src: np.ndarray) -> np.ndarray:
    rows = nb * W
    full = np.empty(B, dtype=np.float32)
    for c in range(NCORES):
        res = np.asarray(results[c]["out"], dtype=np.float32)  # [128, nb]
        flat = res.T.reshape(-1)  # j order: j = 128*b + i
        src = row_src[c * rows : (c + 1) * rows]
        m = src >= 0
        full[src[m]] = flat[m]
    return full


def kernel(**inputs) -> np.ndarray:
    in_maps, nb, row_src = make_in_maps(inputs)
    nc = build_program(nb)
    res = run_bass_kernel_spmd(nc, in_maps, list(range(NCORES)))
    return assemble_output(res.results, nb, row_src)


# revision 6
# speedup vs baseline: 1.9371x; 1.0124x over previous
"""DistMult scoring kernel for Trainium2 (8 NeuronCores, Bass/Tile).

reference computation:
    rel = rel_embeds[rel_ids]                      # [B, D] gather
    scores = sum(head * rel * tail, axis=-1)       # [B]
    pos = min(scores[:n_pos], upper_bound)
    neg = max(scores[n_pos:], lower_bound)
    out = sigmoid(concat(pos, neg))

Strategy (matmul-scored, sorted batches):
  * Host sorts all B rows by rel_id and pads each rel group to a multiple
    of 128 rows, so every 128-row "batch" uses exactly ONE rel vector.
    Batches are dealt contiguously to 8 cores; outputs are unpermuted on
    the host (row order is free to choose since the final gather is ours).
  * Rows are laid out TRANSPOSED on device: d on partitions (2 chunks of
    128), rows on the free axis. h and t are stored int8 (x32) in DRAM and
    upcast to bf16 during the SWDGE DMA (halves HBM traffic; DVE needs
    16-bit for its 2x mode).
  * DVE does a single tensor_tensor pass q = h*t (bf16, 2x mode).
  * Per batch b, the score reduction over d is TWO matmuls on the (idle)
    tensor engine: lhsT = q_chunk[:, b*128:(b+1)*128] (stationary),
    rhs = g[:, 2b+c] (the batch's rel vector chunk, moving, N=1),
    accumulating into psum[:, b]. Measured issue rate ~33ns per pair.
  * psum[i, b] = 1024 * score(row 128b + i). Clamp against host-prescaled
    bounds (x1024, +/-inf on the inactive side), then sigmoid via the
    scalar engine with scale=2^-10. One output DMA; host unpermutes.
"""

import sys

for _p in ("/opt/trn_rl_repo",):
    if _p not in sys.path:
        sys.path.insert(0, _p)

import numpy as np

import concourse.bacc as bacc
import concourse.bass as bass
import concourse.mybir as mybir
import concourse.tile as tile
from concourse.bass_utils import run_bass_kernel_spmd

N_POS = 131072
N_NEG = 393216
B = N_POS + N_NEG  # 524288
D = 256
NUM_REL = 500
NCORES = 8
P = 128
W = 128          # rows per batch (one rel vector per batch)
GB = 48          # batches per group (48*128 = 6144 rows per group-chunk tile)
POOL_B = 14      # batches per group whose h*t runs on GPSIMD (rest on DVE)
HSCALE = 32.0    # h, t int8 quantization scale; scores come out x1024
SSCALE = float(HSCALE * HSCALE)


def build_program(nb: int):
    """nb: batches per core (each 128 rows, single rel)."""
    f32 = mybir.dt.float32
    i8 = mybir.dt.int8
    bf = mybir.dt.bfloat16
    mult = mybir.AluOpType.mult

    rows = nb * W
    ngroups = (nb + GB - 1) // GB

    nc = bacc.Bacc(
        "TRN2", target_bir_lowering=False, debug=False, num_devices=NCORES
    )
    # transposed int8 streams: [chunk, d, row]
    h8 = nc.declare_dram_parameter("h8", [2, P, rows], i8, isOutput=False)
    t8 = nc.declare_dram_parameter("t8", [2, P, rows], i8, isOutput=False)
    # per-batch rel vectors: g[d, 2b+c] = rel_vec(b)[128c + d]
    g = nc.declare_dram_parameter("g", [P, 2 * nb], bf, isOutput=False)
    ub = nc.declare_dram_parameter("ub", [P, nb], f32, isOutput=False)
    lb = nc.declare_dram_parameter("lb", [P, nb], f32, isOutput=False)
    out = nc.declare_dram_parameter("out", [P, nb], f32, isOutput=True)

    with tile.TileContext(nc) as tc:
        with (
            tc.tile_pool(name="io", bufs=1) as io_pool,
            tc.tile_pool(name="stream", bufs=3) as spool,
            tc.tile_pool(name="psum", bufs=4, space="PSUM") as psum_pool,
            tc.tile_pool(name="scratch", bufs=2) as qpool,
        ):
            gt = io_pool.tile([P, 2 * nb], bf)
            nc.sync.dma_start(out=gt[:], in_=g[:])
            ubt = io_pool.tile([P, nb], f32)
            nc.sync.dma_start(out=ubt[:], in_=ub[:])
            lbt = io_pool.tile([P, nb], f32)
            nc.sync.dma_start(out=lbt[:], in_=lb[:])
            scores = io_pool.tile([P, nb], f32)

            for gi in range(ngroups):
                b0 = gi * GB
                gb = min(GB, nb - b0)
                r0 = b0 * W
                gw = gb * W
                # DVE computes h*t for batches [0, sb); pool for [sb, gb)
                sb = max(0, gb - POOL_B)
                sw = sb * W
                ht = [None, None]
                tt = [None, None]
                qt = [None, None]
                for c in range(2):
                    ht[c] = spool.tile([P, GB * W], i8, tag=f"h{c}", name=f"ht{c}")
                    tt[c] = spool.tile([P, GB * W], i8, tag=f"t{c}", name=f"tt{c}")
                    qt[c] = spool.tile([P, GB * W], bf, tag=f"q{c}", name=f"qt{c}")
                    nc.sync.dma_start(
                        out=ht[c][:, :gw], in_=h8[c, :, r0 : r0 + gw]
                    )
                    nc.sync.dma_start(
                        out=tt[c][:, :gw], in_=t8[c, :, r0 : r0 + gw]
                    )
                # q = h * t (int8 x int8 -> bf16): DVE slice + pool slice
                for c in range(2):
                    if sb > 0:
                        nc.vector.tensor_tensor(
                            out=qt[c][:, :sw], in0=ht[c][:, :sw],
                            in1=tt[c][:, :sw], op=mult,
                        )
                    if gb > sb:
                        nc.gpsimd.tensor_tensor(
                            out=qt[c][:, sw:gw], in0=ht[c][:, sw:gw],
                            in1=tt[c][:, sw:gw], op=mult,
                        )
                # per-batch score reduction on the tensor engine
                ps = psum_pool.tile([P, GB], f32, tag="ps")
                for b in range(gb):
                    for c in range(2):
                        nc.tensor.matmul(
                            out=ps[:, b : b + 1],
                            lhsT=qt[c][:, b * W : (b + 1) * W],
                            rhs=gt[:, 2 * (b0 + b) + c : 2 * (b0 + b) + c + 1],
                            start=(c == 0),
                            stop=(c == 1),
                        )
                # clamp into the persistent scores tile
                clip = qpool.tile([P, GB], f32, tag="clip")
                nc.vector.tensor_tensor(
                    out=clip[:, :gb], in0=ps[:, :gb], in1=ubt[:, b0 : b0 + gb],
                    op=mybir.AluOpType.min,
                )
                nc.vector.tensor_tensor(
                    out=scores[:, b0 : b0 + gb], in0=clip[:, :gb],
                    in1=lbt[:, b0 : b0 + gb], op=mybir.AluOpType.max,
                )

            sig = io_pool.tile([P, nb], f32)
            nc.scalar.activation(
                out=sig[:], in_=scores[:],
                func=mybir.ActivationFunctionType.Sigmoid,
                scale=1.0 / SSCALE,
            )
            nc.sync.dma_start(out=out[:], in_=sig[:])

    nc.compile()
    return nc


def make_in_maps(inputs: dict):
    import ml_dtypes

    bf16 = ml_dtypes.bfloat16

    head = np.asarray(inputs["head_embeds"], dtype=np.float32)
    tail = np.asarray(inputs["tail_embeds"], dtype=np.float32)
    rel_ids = np.asarray(inputs["rel_ids"]).astype(np.int64)
    lower = np.asarray(inputs["lower_bound"], dtype=np.float32)
    upper = np.asarray(inputs["upper_bound"], dtype=np.float32)
    table = np.asarray(inputs["rel_embeds"], dtype=np.float32)

    # --- sort rows by rel id, pad each rel group to a multiple of W rows
    order = np.argsort(rel_ids, kind="stable")
    sorted_ids = rel_ids[order]
    counts = np.bincount(sorted_ids, minlength=NUM_REL)
    padded = ((counts + W - 1) // W) * W
    total_batches = int(padded.sum()) // W
    nb = -(-total_batches // NCORES)  # ceil
    nbatch_total = nb * NCORES

    # row_src[j] = original row index or -1 (pad); batch_rel[b] = rel id
    row_src = np.full(nbatch_total * W, -1, dtype=np.int64)
    batch_rel = np.zeros(nbatch_total, dtype=np.int64)
    src_ofs = 0
    dst_ofs = 0
    bidx = 0
    for k in range(NUM_REL):
        n = int(counts[k])
        pn = int(padded[k])
        if pn == 0:
            continue
        row_src[dst_ofs : dst_ofs + n] = order[src_ofs : src_ofs + n]
        batch_rel[bidx : bidx + pn // W] = k
        src_ofs += n
        dst_ofs += pn
        bidx += pn // W

    # --- quantize h, t to int8 (x32) and build per-core transposed streams
    h8 = np.clip(np.round(head * HSCALE), -127, 127).astype(np.int8)
    t8 = np.clip(np.round(tail * HSCALE), -127, 127).astype(np.int8)
    src = row_src.copy()
    pad_mask = src < 0
    src[pad_mask] = 0
    h8_s = h8[src]
    t8_s = t8[src]
    h8_s[pad_mask] = 0
    t8_s[pad_mask] = 0

    # bounds in score space (x1024), +/-inf on the inactive side
    ubf = np.full(nbatch_total * W, np.inf, dtype=np.float32)
    lbf = np.full(nbatch_total * W, -np.inf, dtype=np.float32)
    pos_rows = (row_src >= 0) & (row_src < N_POS)
    neg_rows = row_src >= N_POS
    ubf[pos_rows] = upper[row_src[pos_rows]] * SSCALE
    lbf[neg_rows] = lower[row_src[neg_rows] - N_POS] * SSCALE

    table_bf = table.astype(bf16)

    rows = nb * W
    in_maps = []
    for c in range(NCORES):
        r0 = c * rows
        r1 = r0 + rows
        # [rows, 256] -> [256, rows] -> [2, 128, rows]
        hc = np.ascontiguousarray(h8_s[r0:r1].T).reshape(2, P, rows)
        tc_ = np.ascontiguousarray(t8_s[r0:r1].T).reshape(2, P, rows)
        # g[d, 2b+c] = table[rel(b), 128c+d]
        rel_c = batch_rel[c * nb : (c + 1) * nb]
        gc = table_bf[rel_c].reshape(nb, 2, P)  # [b, chunk, d]
        gc = np.ascontiguousarray(gc.transpose(2, 0, 1).reshape(P, 2 * nb))
        # bounds laid [i, b]: row j = 128b + i
        ub_c = np.ascontiguousarray(ubf[r0:r1].reshape(nb, W).T)
        lb_c = np.ascontiguousarray(lbf[r0:r1].reshape(nb, W).T)
        in_maps.append(
            {"h8": hc, "t8": tc_, "g": gc, "ub": ub_c, "lb": lb_c}
        )
    return in_maps, nb, row_src


def assemble_output(results, nb: int, row_src: np.ndarray) -> np.ndarray:
    rows = nb * W
    full = np.empty(B, dtype=np.float32)
    for c in range(NCORES):
        res = np.asarray(results[c]["out"], dtype=np.float32)  # [128, nb]
        flat = res.T.reshape(-1)  # j order: j = 128*b + i
        src = row_src[c * rows : (c + 1) * rows]
        m = src >= 0
        full[src[m]] = flat[m]
    return full


def kernel(**inputs) -> np.ndarray:
    in_maps, nb, row_src = make_in_maps(inputs)
    nc = build_program(nb)
    res = run_bass_kernel_spmd(nc, in_maps, list(range(NCORES)))
    return assemble_output(res.results, nb, row_src)


# revision 8
# speedup vs baseline: 2.0127x; 1.0390x over previous
"""DistMult scoring kernel for Trainium2 (8 NeuronCores, Bass/Tile).

reference computation:
    rel = rel_embeds[rel_ids]                      # [B, D] gather
    scores = sum(head * rel * tail, axis=-1)       # [B]
    pos = min(scores[:n_pos], upper_bound)
    neg = max(scores[n_pos:], lower_bound)
    out = sigmoid(concat(pos, neg))

Strategy (matmul-scored, sorted batches):
  * Host sorts all B rows by rel_id and pads each rel group to a multiple
    of 128 rows, so every 128-row "batch" uses exactly ONE rel vector.
    Batches are dealt contiguously to 8 cores; outputs are unpermuted on
    the host (row order is free to choose since the final gather is ours).
  * Rows are laid out TRANSPOSED on device: d on partitions (2 chunks of
    128), rows on the free axis. h and t are stored int8 (x32) in DRAM and
    upcast to bf16 during the SWDGE DMA (halves HBM traffic; DVE needs
    16-bit for its 2x mode).
  * DVE does a single tensor_tensor pass q = h*t (bf16, 2x mode).
  * Per batch b, the score reduction over d is TWO matmuls on the (idle)
    tensor engine: lhsT = q_chunk[:, b*128:(b+1)*128] (stationary),
    rhs = g[:, 2b+c] (the batch's rel vector chunk, moving, N=1),
    accumulating into psum[:, b]. Measured issue rate ~33ns per pair.
  * psum[i, b] = 1024 * score(row 128b + i). Clamp against host-prescaled
    bounds (x1024, +/-inf on the inactive side), then sigmoid via the
    scalar engine with scale=2^-10. One output DMA; host unpermutes.
"""

import sys

for _p in ("/opt/trn_rl_repo",):
    if _p not in sys.path:
        sys.path.insert(0, _p)

import numpy as np

import concourse.bacc as bacc
import concourse.bass as bass
import concourse.mybir as mybir
import concourse.tile as tile
from concourse.bass_utils import run_bass_kernel_spmd

N_POS = 131072
N_NEG = 393216
B = N_POS + N_NEG  # 524288
D = 256
NUM_REL = 500
NCORES = 8
P = 128
W = 128          # rows per batch (one rel vector per batch)
GB = 48          # batches per group (48*128 = 6144 rows per group-chunk tile)
POOL_B = 8       # batches per group on the raw-int8 GPSIMD path (rest: cast+DVE)
HSCALE = 32.0    # h, t int8 quantization scale; scores come out x1024
SSCALE = float(HSCALE * HSCALE)


def build_program(nb: int):
    """nb: batches per core (each 128 rows, single rel)."""
    f32 = mybir.dt.float32
    i8 = mybir.dt.int8
    bf = mybir.dt.bfloat16
    mult = mybir.AluOpType.mult

    rows = nb * W
    ngroups = (nb + GB - 1) // GB

    nc = bacc.Bacc(
        "TRN2", target_bir_lowering=False, debug=False, num_devices=NCORES
    )
    # transposed int8 streams: [chunk, d, row]
    h8 = nc.declare_dram_parameter("h8", [2, P, rows], i8, isOutput=False)
    t8 = nc.declare_dram_parameter("t8", [2, P, rows], i8, isOutput=False)
    # per-batch rel vectors: g[d, 2b+c] = rel_vec(b)[128c + d]
    g = nc.declare_dram_parameter("g", [P, 2 * nb], bf, isOutput=False)
    ub = nc.declare_dram_parameter("ub", [P, nb], f32, isOutput=False)
    lb = nc.declare_dram_parameter("lb", [P, nb], f32, isOutput=False)
    out = nc.declare_dram_parameter("out", [P, nb], f32, isOutput=True)

    with tile.TileContext(nc) as tc:
        with (
            tc.tile_pool(name="io", bufs=1) as io_pool,
            tc.tile_pool(name="stream", bufs=3) as spool,
            tc.tile_pool(name="psum", bufs=4, space="PSUM") as psum_pool,
            tc.tile_pool(name="scratch", bufs=2) as qpool,
        ):
            gt = io_pool.tile([P, 2 * nb], bf)
            nc.sync.dma_start(out=gt[:], in_=g[:])
            ubt = io_pool.tile([P, nb], f32)
            nc.sync.dma_start(out=ubt[:], in_=ub[:])
            lbt = io_pool.tile([P, nb], f32)
            nc.sync.dma_start(out=lbt[:], in_=lb[:])
            scores = io_pool.tile([P, nb], f32)

            for gi in range(ngroups):
                b0 = gi * GB
                gb = min(GB, nb - b0)
                r0 = b0 * W
                gw = gb * W
                # batches [0, sb): int8 cast-DMA -> bf16 tiles, DVE TT (2x mode)
                # batches [sb, gb): raw int8 HWDGE loads, GPSIMD TT
                sb = max(0, gb - POOL_B)
                sw = sb * W
                SW = (GB - POOL_B) * W
                PW = POOL_B * W
                hb = [None, None]
                tb = [None, None]
                h8t = [None, None]
                t8t = [None, None]
                qp = [None, None]
                for c in range(2):
                    hb[c] = spool.tile([P, SW], bf, tag=f"h{c}", name=f"hb{c}")
                    tb[c] = spool.tile([P, SW], bf, tag=f"t{c}", name=f"tb{c}")
                    if sb > 0:
                        nc.gpsimd.dma_start(
                            out=hb[c][:, :sw], in_=h8[c, :, r0 : r0 + sw]
                        )
                        nc.gpsimd.dma_start(
                            out=tb[c][:, :sw], in_=t8[c, :, r0 : r0 + sw]
                        )
                    h8t[c] = spool.tile([P, PW], i8, tag=f"h8{c}", name=f"h8t{c}")
                    t8t[c] = spool.tile([P, PW], i8, tag=f"t8{c}", name=f"t8t{c}")
                    qp[c] = spool.tile([P, PW], bf, tag=f"qp{c}", name=f"qp{c}")
                    if gb > sb:
                        pw = (gb - sb) * W
                        nc.sync.dma_start(
                            out=h8t[c][:, :pw], in_=h8[c, :, r0 + sw : r0 + gw]
                        )
                        nc.sync.dma_start(
                            out=t8t[c][:, :pw], in_=t8[c, :, r0 + sw : r0 + gw]
                        )
                for c in range(2):
                    if sb > 0:
                        nc.vector.tensor_tensor(
                            out=hb[c][:, :sw], in0=hb[c][:, :sw],
                            in1=tb[c][:, :sw], op=mult,
                        )
                    if gb > sb:
                        pw = (gb - sb) * W
                        nc.gpsimd.tensor_tensor(
                            out=qp[c][:, :pw], in0=h8t[c][:, :pw],
                            in1=t8t[c][:, :pw], op=mult,
                        )
                # per-batch score reduction on the tensor engine
                ps = psum_pool.tile([P, GB], f32, tag="ps")
                for b in range(gb):
                    for c in range(2):
                        if b < sb:
                            lhsT = hb[c][:, b * W : (b + 1) * W]
                        else:
                            lhsT = qp[c][:, (b - sb) * W : (b - sb + 1) * W]
                        nc.tensor.matmul(
                            out=ps[:, b : b + 1],
                            lhsT=lhsT,
                            rhs=gt[:, 2 * (b0 + b) + c : 2 * (b0 + b) + c + 1],
                            start=(c == 0),
                            stop=(c == 1),
                        )
                # clamp into the persistent scores tile
                clip = qpool.tile([P, GB], f32, tag="clip")
                nc.vector.tensor_tensor(
                    out=clip[:, :gb], in0=ps[:, :gb], in1=ubt[:, b0 : b0 + gb],
                    op=mybir.AluOpType.min,
                )
                nc.vector.tensor_tensor(
                    out=scores[:, b0 : b0 + gb], in0=clip[:, :gb],
                    in1=lbt[:, b0 : b0 + gb], op=mybir.AluOpType.max,
                )

            sig = io_pool.tile([P, nb], f32)
            nc.scalar.activation(
                out=sig[:], in_=scores[:],
                func=mybir.ActivationFunctionType.Sigmoid,
                scale=1.0 / SSCALE,
            )
            nc.sync.dma_start(out=out[:], in_=sig[:])

    nc.compile()
    return nc


def make_in_maps(inputs: dict):
    import ml_dtypes

    bf16 = ml_dtypes.bfloat16

    head = np.asarray(inputs["head_embeds"], dtype=np.float32)
    tail = np.asarray(inputs["tail_embeds"], dtype=np.float32)
    rel_ids = np.asarray(inputs["rel_ids"]).astype(np.int64)
    lower = np.asarray(inputs["lower_bound"], dtype=np.float32)
    upper = np.asarray(inputs["upper_bound"], dtype=np.float32)
    table = np.asarray(inputs["rel_embeds"], dtype=np.float32)

    # --- sort rows by rel id, pad each rel group to a multiple of W rows
    order = np.argsort(rel_ids, kind="stable")
    sorted_ids = rel_ids[order]
    counts = np.bincount(sorted_ids, minlength=NUM_REL)
    padded = ((counts + W - 1) // W) * W
    total_batches = int(padded.sum()) // W
    nb = -(-total_batches // NCORES)  # ceil
    nbatch_total = nb * NCORES

    # row_src[j] = original row index or -1 (pad); batch_rel[b] = rel id
    row_src = np.full(nbatch_total * W, -1, dtype=np.int64)
    batch_rel = np.zeros(nbatch_total, dtype=np.int64)
    src_ofs = 0
    dst_ofs = 0
    bidx = 0
    for k in range(NUM_REL):
        n = int(counts[k])
        pn = int(padded[k])
        if pn == 0:
            continue
        row_src[dst_ofs : dst_ofs + n] = order[src_ofs : src_ofs + n]
        batch_rel[bidx : bidx + pn // W] = k
        src_ofs += n
        dst_ofs += pn
        bidx += pn // W

    # --- quantize h, t to int8 (x32) and build per-core transposed streams
    h8 = np.clip(np.round(head * HSCALE), -127, 127).astype(np.int8)
    t8 = np.clip(np.round(tail * HSCALE), -127, 127).astype(np.int8)
    src = row_src.copy()
    pad_mask = src < 0
    src[pad_mask] = 0
    h8_s = h8[src]
    t8_s = t8[src]
    h8_s[pad_mask] = 0
    t8_s[pad_mask] = 0

    # bounds in score space (x1024), +/-inf on the inactive side
    ubf = np.full(nbatch_total * W, np.inf, dtype=np.float32)
    lbf = np.full(nbatch_total * W, -np.inf, dtype=np.float32)
    pos_rows = (row_src >= 0) & (row_src < N_POS)
    neg_rows = row_src >= N_POS
    ubf[pos_rows] = upper[row_src[pos_rows]] * SSCALE
    lbf[neg_rows] = lower[row_src[neg_rows] - N_POS] * SSCALE

    table_bf = table.astype(bf16)

    rows = nb * W
    in_maps = []
    for c in range(NCORES):
        r0 = c * rows
        r1 = r0 + rows
        # [rows, 256] -> [256, rows] -> [2, 128, rows]
        hc = np.ascontiguousarray(h8_s[r0:r1].T).reshape(2, P, rows)
        tc_ = np.ascontiguousarray(t8_s[r0:r1].T).reshape(2, P, rows)
        # g[d, 2b+c] = table[rel(b), 128c+d]
        rel_c = batch_rel[c * nb : (c + 1) * nb]
        gc = table_bf[rel_c].reshape(nb, 2, P)  # [b, chunk, d]
        gc = np.ascontiguousarray(gc.transpose(2, 0, 1).reshape(P, 2 * nb))
        # bounds laid [i, b]: row j = 128b + i
        ub_c = np.ascontiguousarray(ubf[r0:r1].reshape(nb, W).T)
        lb_c = np.ascontiguousarray(lbf[r0:r1].reshape(nb, W).T)
        in_maps.append(
            {"h8": hc, "t8": tc_, "g": gc, "ub": ub_c, "lb": lb_c}
        )
    return in_maps, nb, row_src


def assemble_output(results, nb: int, row_src: np.ndarray) -> np.ndarray:
    rows = nb * W
    full = np.empty(B, dtype=np.float32)
    for c in range(NCORES):
        res = np.asarray(results[c]["out"], dtype=np.float32)  # [128, nb]
        flat = res.T.reshape(-1)  # j order: j = 128*b + i
        src = row_src[c * rows : (c + 1) * rows]
        m = src >= 0
        full[src[m]] = flat[m]
    return full


def kernel(**inputs) -> np.ndarray:
    in_maps, nb, row_src = make_in_maps(inputs)
    nc = build_program(nb)
    res = run_bass_kernel_spmd(nc, in_maps, list(range(NCORES)))
    return assemble_output(res.results, nb, row_src)


# revision 11
# speedup vs baseline: 2.0839x; 1.0354x over previous
"""DistMult scoring kernel for Trainium2 (8 NeuronCores, Bass/Tile).

reference computation:
    rel = rel_embeds[rel_ids]                      # [B, D] gather
    scores = sum(head * rel * tail, axis=-1)       # [B]
    pos = min(scores[:n_pos], upper_bound)
    neg = max(scores[n_pos:], lower_bound)
    out = sigmoid(concat(pos, neg))

Strategy (matmul-scored, sorted batches):
  * Host sorts all B rows by rel_id and pads each rel group to a multiple
    of 128 rows, so every 128-row "batch" uses exactly ONE rel vector.
    Batches are dealt contiguously to 8 cores; outputs are unpermuted on
    the host (row order is free to choose since the final gather is ours).
  * Rows are laid out TRANSPOSED on device: d on partitions (2 chunks of
    128), rows on the free axis. h and t are stored int8 (x32) in DRAM and
    upcast to bf16 during the SWDGE DMA (halves HBM traffic; DVE needs
    16-bit for its 2x mode).
  * DVE does a single tensor_tensor pass q = h*t (bf16, 2x mode).
  * Per batch b, the score reduction over d is TWO matmuls on the (idle)
    tensor engine: lhsT = q_chunk[:, b*128:(b+1)*128] (stationary),
    rhs = g[:, 2b+c] (the batch's rel vector chunk, moving, N=1),
    accumulating into psum[:, b]. Measured issue rate ~33ns per pair.
  * psum[i, b] = 1024 * score(row 128b + i). Clamp against host-prescaled
    bounds (x1024, +/-inf on the inactive side), then sigmoid via the
    scalar engine with scale=2^-10. One output DMA; host unpermutes.
"""

import sys

for _p in ("/opt/trn_rl_repo",):
    if _p not in sys.path:
        sys.path.insert(0, _p)

import numpy as np

import concourse.bacc as bacc
import concourse.bass as bass
import concourse.mybir as mybir
import concourse.tile as tile
from concourse.bass_utils import run_bass_kernel_spmd

N_POS = 131072
N_NEG = 393216
B = N_POS + N_NEG  # 524288
D = 256
NUM_REL = 500
NCORES = 8
P = 128
W = 128          # rows per batch (one rel vector per batch)
GB = 48          # batches per group (48*128 = 6144 rows per group-chunk tile)
CAST_B = 24      # batches per group on the cast-DMA + DVE-bf16 (2x) path
DVE8_B = 17      # batches per group on the raw-int8 DVE (1x) path
POOL_B = 7       # batches per group on the raw-int8 GPSIMD path
HSCALE = 32.0    # h, t int8 quantization scale; scores come out x1024
SSCALE = float(HSCALE * HSCALE)


def build_program(nb: int):
    """nb: batches per core (each 128 rows, single rel)."""
    f32 = mybir.dt.float32
    i8 = mybir.dt.int8
    bf = mybir.dt.bfloat16
    mult = mybir.AluOpType.mult

    rows = nb * W
    ngroups = (nb + GB - 1) // GB

    nc = bacc.Bacc(
        "TRN2", target_bir_lowering=False, debug=False, num_devices=NCORES
    )
    # transposed int8 streams: [chunk, d, row]
    h8 = nc.declare_dram_parameter("h8", [2, P, rows], i8, isOutput=False)
    t8 = nc.declare_dram_parameter("t8", [2, P, rows], i8, isOutput=False)
    # per-batch rel vectors: g[d, 2b+c] = rel_vec(b)[128c + d]
    g = nc.declare_dram_parameter("g", [P, 2 * nb], bf, isOutput=False)
    ub = nc.declare_dram_parameter("ub", [P, nb], f32, isOutput=False)
    lb = nc.declare_dram_parameter("lb", [P, nb], f32, isOutput=False)
    out = nc.declare_dram_parameter("out", [P, nb], f32, isOutput=True)

    with tile.TileContext(nc) as tc:
        with (
            tc.tile_pool(name="io", bufs=1) as io_pool,
            tc.tile_pool(name="stream", bufs=3) as spool,
            tc.tile_pool(name="psum", bufs=4, space="PSUM") as psum_pool,
            tc.tile_pool(name="scratch", bufs=2) as qpool,
        ):
            gt = io_pool.tile([P, 2 * nb], bf)
            nc.sync.dma_start(out=gt[:], in_=g[:])
            ubt = io_pool.tile([P, nb], f32)
            nc.sync.dma_start(out=ubt[:], in_=ub[:])
            lbt = io_pool.tile([P, nb], f32)
            nc.sync.dma_start(out=lbt[:], in_=lb[:])
            scores = io_pool.tile([P, nb], f32)

            # group size schedule: small first groups shorten the pipeline head
            sizes = []
            remaining = nb
            for sz in (16, 32):
                if remaining > sz:
                    sizes.append(sz)
                    remaining -= sz
            while remaining > 0:
                sz = min(GB, remaining)
                sizes.append(sz)
                remaining -= sz

            CW = CAST_B * W
            IW = (DVE8_B + POOL_B) * W
            b0 = 0
            for gi, gb in enumerate(sizes):
                r0 = b0 * W
                gw = gb * W
                # per-group 3-way split, scaled to group size
                cb = (gb * CAST_B) // GB
                db = (gb * DVE8_B) // GB
                pb = gb - cb - db
                cw = cb * W
                dw = db * W
                iw = (db + pb) * W
                hb = [None, None]
                tb = [None, None]
                h8t = [None, None]
                t8t = [None, None]
                qp = [None, None]
                for c in range(2):
                    hb[c] = spool.tile([P, CW], bf, tag=f"h{c}", name=f"hb{c}")
                    tb[c] = spool.tile([P, CW], bf, tag=f"t{c}", name=f"tb{c}")
                    if cb > 0:
                        nc.gpsimd.dma_start(
                            out=hb[c][:, :cw], in_=h8[c, :, r0 : r0 + cw]
                        )
                        nc.gpsimd.dma_start(
                            out=tb[c][:, :cw], in_=t8[c, :, r0 : r0 + cw]
                        )
                    h8t[c] = spool.tile([P, IW], i8, tag=f"h8{c}", name=f"h8t{c}")
                    t8t[c] = spool.tile([P, IW], i8, tag=f"t8{c}", name=f"t8t{c}")
                    qp[c] = spool.tile([P, IW], bf, tag=f"qp{c}", name=f"qp{c}")
                    if iw > 0:
                        nc.sync.dma_start(
                            out=h8t[c][:, :iw], in_=h8[c, :, r0 + cw : r0 + gw]
                        )
                        nc.sync.dma_start(
                            out=t8t[c][:, :iw], in_=t8[c, :, r0 + cw : r0 + gw]
                        )
                for c in range(2):
                    if cb > 0:
                        nc.vector.tensor_tensor(
                            out=hb[c][:, :cw], in0=hb[c][:, :cw],
                            in1=tb[c][:, :cw], op=mult,
                        )
                    if db > 0:
                        nc.vector.tensor_tensor(
                            out=qp[c][:, :dw], in0=h8t[c][:, :dw],
                            in1=t8t[c][:, :dw], op=mult,
                        )
                    if pb > 0:
                        nc.gpsimd.tensor_tensor(
                            out=qp[c][:, dw:iw], in0=h8t[c][:, dw:iw],
                            in1=t8t[c][:, dw:iw], op=mult,
                        )
                # per-batch score reduction on the tensor engine
                ps = psum_pool.tile([P, GB], f32, tag="ps")
                for b in range(gb):
                    for c in range(2):
                        if b < cb:
                            lhsT = hb[c][:, b * W : (b + 1) * W]
                        else:
                            lhsT = qp[c][:, (b - cb) * W : (b - cb + 1) * W]
                        nc.tensor.matmul(
                            out=ps[:, b : b + 1],
                            lhsT=lhsT,
                            rhs=gt[:, 2 * (b0 + b) + c : 2 * (b0 + b) + c + 1],
                            start=(c == 0),
                            stop=(c == 1),
                        )
                # clamp into the persistent scores tile
                clip = qpool.tile([P, GB], f32, tag="clip")
                nc.vector.tensor_tensor(
                    out=clip[:, :gb], in0=ps[:, :gb], in1=ubt[:, b0 : b0 + gb],
                    op=mybir.AluOpType.min,
                )
                nc.vector.tensor_tensor(
                    out=scores[:, b0 : b0 + gb], in0=clip[:, :gb],
                    in1=lbt[:, b0 : b0 + gb], op=mybir.AluOpType.max,
                )
                b0 += gb

            sig = io_pool.tile([P, nb], f32)
            nc.scalar.activation(
                out=sig[:], in_=scores[:],
                func=mybir.ActivationFunctionType.Sigmoid,
                scale=1.0 / SSCALE,
            )
            nc.sync.dma_start(out=out[:], in_=sig[:])

    nc.compile()
    return nc


def make_in_maps(inputs: dict):
    import ml_dtypes

    bf16 = ml_dtypes.bfloat16

    head = np.asarray(inputs["head_embeds"], dtype=np.float32)
    tail = np.asarray(inputs["tail_embeds"], dtype=np.float32)
    rel_ids = np.asarray(inputs["rel_ids"]).astype(np.int64)
    lower = np.asarray(inputs["lower_bound"], dtype=np.float32)
    upper = np.asarray(inputs["upper_bound"], dtype=np.float32)
    table = np.asarray(inputs["rel_embeds"], dtype=np.float32)

    # --- sort rows by rel id, pad each rel group to a multiple of W rows
    order = np.argsort(rel_ids, kind="stable")
    sorted_ids = rel_ids[order]
    counts = np.bincount(sorted_ids, minlength=NUM_REL)
    padded = ((counts + W - 1) // W) * W
    total_batches = int(padded.sum()) // W
    nb = -(-total_batches // NCORES)  # ceil
    nbatch_total = nb * NCORES

    # row_src[j] = original row index or -1 (pad); batch_rel[b] = rel id
    row_src = np.full(nbatch_total * W, -1, dtype=np.int64)
    batch_rel = np.zeros(nbatch_total, dtype=np.int64)
    src_ofs = 0
    dst_ofs = 0
    bidx = 0
    for k in range(NUM_REL):
        n = int(counts[k])
        pn = int(padded[k])
        if pn == 0:
            continue
        row_src[dst_ofs : dst_ofs + n] = order[src_ofs : src_ofs + n]
        batch_rel[bidx : bidx + pn // W] = k
        src_ofs += n
        dst_ofs += pn
        bidx += pn // W

    # --- quantize h, t to int8 (x32) and build per-core transposed streams
    h8 = np.clip(np.round(head * HSCALE), -127, 127).astype(np.int8)
    t8 = np.clip(np.round(tail * HSCALE), -127, 127).astype(np.int8)
    src = row_src.copy()
    pad_mask = src < 0
    src[pad_mask] = 0
    h8_s = h8[src]
    t8_s = t8[src]
    h8_s[pad_mask] = 0
    t8_s[pad_mask] = 0

    # bounds in score space (x1024), +/-inf on the inactive side
    ubf = np.full(nbatch_total * W, np.inf, dtype=np.float32)
    lbf = np.full(nbatch_total * W, -np.inf, dtype=np.float32)
    pos_rows = (row_src >= 0) & (row_src < N_POS)
    neg_rows = row_src >= N_POS
    ubf[pos_rows] = upper[row_src[pos_rows]] * SSCALE
    lbf[neg_rows] = lower[row_src[neg_rows] - N_POS] * SSCALE

    table_bf = table.astype(bf16)

    rows = nb * W
    in_maps = []
    for c in range(NCORES):
        r0 = c * rows
        r1 = r0 + rows
        # [rows, 256] -> [256, rows] -> [2, 128, rows]
        hc = np.ascontiguousarray(h8_s[r0:r1].T).reshape(2, P, rows)
        tc_ = np.ascontiguousarray(t8_s[r0:r1].T).reshape(2, P, rows)
        # g[d, 2b+c] = table[rel(b), 128c+d]
        rel_c = batch_rel[c * nb : (c + 1) * nb]
        gc = table_bf[rel_c].reshape(nb, 2, P)  # [b, chunk, d]
        gc = np.ascontiguousarray(gc.transpose(2, 0, 1).reshape(P, 2 * nb))
        # bounds laid [i, b]: row j = 128b + i
        ub_c = np.ascontiguousarray(ubf[r0:r1].reshape(nb, W).T)
        lb_c = np.ascontiguousarray(lbf[r0:r1].reshape(nb, W).T)
        in_maps.append(
            {"h8": hc, "t8": tc_, "g": gc, "ub": ub_c, "lb": lb_c}
        )
    return in_maps, nb, row_src


def assemble_output(results, nb: int, row_src: np.ndarray) -> np.ndarray:
    rows = nb * W
    full = np.empty(B, dtype=np.float32)
    for c in range(NCORES):
        res = np.asarray(results[c]["out"], dtype=np.float32)  # [128, nb]
        flat = res.T.reshape(-1)  # j order: j = 128*b + i
        src = row_src[c * rows : (c + 1) * rows]
        m = src >= 0
        full[src[m]] = flat[m]
    return full


def kernel(**inputs) -> np.ndarray:
    in_maps, nb, row_src = make_in_maps(inputs)
    nc = build_program(nb)
    res = run_bass_kernel_spmd(nc, in_maps, list(range(NCORES)))
    return assemble_output(res.results, nb, row_src)
